# revision 3
# baseline (speedup 1.0000x reference)
"""Bidirectional column-chained GRU (vertical BiGRU over image columns) on 8 Trainium2 cores.

Topology: cores 0-3 run the forward GRU chain (batch quarters), cores 4-7 the
backward chain (rows pre-reversed on host). Each core runs the full C*S=16384
sequential GRU steps for its 8 batch rows in feature-major layout (128
partitions = hidden dim, free dim = batch).

Math restructuring (validated vs reference):
  state hp1 = h + 1; tanh(x) = 2*sigmoid(2x) - 1 (single ACT table).
  Recurrent weights, input columns and the state pair are bf16 (PSUM
  accumulation stays fp32); the serialized fp32 LDWEIGHTS cost dominated the
  per-step critical cycle, and bf16 enables fast weight load.
  Per column c, each gate's rank-1 input contribution is preloaded into PSUM
  with K=2 matmuls; the recurrent matmul accumulates per step into PSUM
  slice t. Per step:
    r  = sigmoid(ps_r[t])                ACT (PSUM src)
    u  = sigmoid(-ps_z[t])  (= 1-z)      ACT
    q  = r * ps_n[t]                     DVE
    w  = q + a_n[t]                      DVE
    v  = sigmoid(2w)                     ACT
    e1 = u * hp1; f = hp1 - e1           DVE (slack path)
    e2 = 2*u*v                           DVE (scalar_tensor_tensor)
    hp1' = f + e2                        DVE (slack path)
  The state pair is stored as [f | e2]; the next step's three matmuls consume
  it directly (PSUM accumulation computes W @ (f + e2) = W @ hp1'), so the
  matmuls never wait for the hp1 update.
  Final per-column features h = hp1 - 1 are collected; the output head
  (fc + relu + softmax) runs on-device with a pairwise AllReduce between the
  fwd/bwd core of each batch group; b_fc/2 is folded into each core's PSUM
  accumulation via a ones-row matmul. exp(relu(x)) == max(1, exp(x)).

Runner: the jitted shard_map callable is built once and cached; repeat
kernel() calls only pay input transfer + execution.
"""

import numpy as np

import jax
from jax.sharding import Mesh, PartitionSpec
from jax.experimental.shard_map import shard_map

import concourse.bass as bass
import concourse.bacc as bacc
import concourse.mybir as mybir
import concourse.tile as tile
from concourse.bass2jax import (
    _bass_exec_p,
    install_neuronx_cc_hook,
    partition_id_tensor,
)

B, S, C, H, O = 32, 128, 128, 128, 64
NCORES = 8
BL = B // 4          # batch rows per core (4 groups x 2 directions)
SB = S * BL          # rhs columns per image column
HS = SB // 2         # half-column psum width (one bank)
NSTEP = S // 2       # steps per half
f32 = mybir.dt.float32
bf16 = mybir.dt.bfloat16
FP = mybir.EngineType

UV_BCAST = False     # measured: [f|e2]+stt beats [uv|uv|f] broadcast-write


def _emit(nc: bacc.Bacc, n_cols: int = C, loop_cols: int | None = None,
          skip_collective: bool = False, sph: int = NSTEP):
    AF = mybir.ActivationFunctionType
    OPM = mybir.AluOpType.mult
    NSL = 3 if UV_BCAST else 2

    xaug_d = nc.dram_tensor("xaug", [n_cols * 2, SB], bf16, kind="ExternalInput").ap()
    hp10_d = nc.dram_tensor("hp10", [H, BL], f32, kind="ExternalInput").ap()
    whhrT_d = nc.dram_tensor("whhrT", [H, H], bf16, kind="ExternalInput").ap()
    whhzT_d = nc.dram_tensor("whhzT", [H, H], bf16, kind="ExternalInput").ap()
    whhnT_d = nc.dram_tensor("whhnT", [H, H], bf16, kind="ExternalInput").ap()
    lcat_d = nc.dram_tensor("lcat", [2, 4 * H], bf16, kind="ExternalInput").ap()
    wfcT_d = nc.dram_tensor("wfcT", [H, O], f32, kind="ExternalInput").ap()
    bias_d = nc.dram_tensor("bias_half", [1, 8 * O], f32, kind="ExternalInput").ap()
    out_d = nc.dram_tensor("out", [C * BL, O], f32, kind="ExternalOutput").ap()

    ncols_eff = n_cols if loop_cols is None else loop_cols

    with tile.TileContext(nc) as tc:
        with tc.tile_pool(name="const", bufs=1) as cp:
            whhrT = cp.tile([H, H], bf16)
            whhzT = cp.tile([H, H], bf16)
            whhnT = cp.tile([H, H], bf16)
            lcat = cp.tile([2, 4 * H], bf16)
            wfcT = cp.tile([H, O], f32)
            biasr = cp.tile([1, 8 * O], f32)
            ones1 = cp.tile([1, H], f32)
            hp1 = cp.tile([H, BL], f32)
            hall = cp.tile([H, C * BL], f32)
            r = cp.tile([H, BL], f32)
            u = cp.tile([H, BL], f32)
            q = cp.tile([H, BL], f32)
            w = cp.tile([H, BL], f32)
            v = cp.tile([H, BL], f32)
            e1 = cp.tile([H, BL], f32)
            fe2 = cp.tile([H, NSL * BL], bf16)
            if UV_BCAST:
                uv = fe2[:, 0:BL]
                fp_ = fe2[:, 2 * BL : 3 * BL]
            else:
                fp_ = fe2[:, 0:BL]
                uv = fe2[:, BL : 2 * BL]

            nc.sync.dma_start(whhrT[:], whhrT_d)
            nc.sync.dma_start(whhzT[:], whhzT_d)
            nc.sync.dma_start(whhnT[:], whhnT_d)
            nc.sync.dma_start(lcat[:], lcat_d)
            nc.sync.dma_start(wfcT[:], wfcT_d)
            nc.sync.dma_start(biasr[:], bias_d)
            nc.sync.dma_start(hp1[:], hp10_d)
            nc.vector.memset(ones1[:], 1.0)
            nc.vector.tensor_scalar_add(fp_[:], hp1[:], 0.0)  # f32->bf16
            if UV_BCAST:
                nc.vector.memzero(fe2[:, 0 : 2 * BL])
            else:
                nc.vector.memzero(uv)

            def column_body(colp, psp, cv):
                xa = colp.tile([2, SB], bf16, tag="xa")
                nc.sync.dma_start(xa[:], xaug_d[bass.ds(cv * 2, 2), :])

                def preload(half):
                    ps_r = psp.tile([H, HS], f32, tag="ps_r", name=f"ps_r{half}")[:]
                    ps_z = psp.tile([H, HS], f32, tag="ps_z", name=f"ps_z{half}")[:]
                    ps_n = psp.tile([H, HS], f32, tag="ps_n", name=f"ps_n{half}")
                    ps_t = psp.tile([H, HS], f32, tag="ps_t", name=f"ps_t{half}")
                    a_n = colp.tile([H, HS], f32, tag="a_n", name=f"a_n{half}")
                    xh = xa[:, half * HS : (half + 1) * HS]
                    lcv = lcat[:]
                    nc.tensor.matmul(ps_r, lcv[:, 0:H], xh, start=True, stop=True)
                    nc.tensor.matmul(ps_z, lcv[:, H : 2 * H], xh, start=True, stop=True)
                    nc.tensor.matmul(ps_n[:], lcv[:, 2 * H : 3 * H], xh, start=True, stop=True)
                    nc.tensor.matmul(ps_t[:], lcv[:, 3 * H : 4 * H], xh, start=True, stop=True)
                    nc.scalar.copy(a_n[:], ps_t[:])
                    return ps_r, ps_z, ps_n, a_n

                def steps(ph, lo, hi):
                    ps_r, ps_z, ps_n, a_n = ph
                    for t in range(lo, hi):
                        sl = slice(t * BL, (t + 1) * BL)
                        hp1v = fe2[:].rearrange("p (a o) -> p a o", a=NSL)
                        mm_order = [(ps_r, whhrT), (ps_n[:], whhnT), (ps_z, whhzT)]
                        outs = [
                            bass.broadcast_tensor_aps(
                                ps[:, sl].rearrange("p (a o) -> p a o", a=1),
                                hp1v,
                            )[0]
                            for ps, _ in mm_order
                        ]
                        for o_, (_, w_) in zip(outs, mm_order):
                            nc.tensor.matmul(
                                o_, w_[:], hp1v, start=False, stop=True,
                                skip_group_check=True,
                            )
                        nc.scalar.activation(r[:], ps_r[:, sl], AF.Sigmoid)
                        nc.scalar.activation(u[:], ps_z[:, sl], AF.Sigmoid, scale=-1.0)
                        rr = r[:]
                        uu = u[:]
                        nc.vector.tensor_mul(q[:], rr, ps_n[:, sl])
                        nc.vector.tensor_add(w[:], q[:], a_n[:, sl])
                        nc.scalar.activation(v[:], w[:], AF.Sigmoid, scale=2.0)
                        nc.vector.tensor_mul(e1[:], uu, hp1[:])
                        nc.vector.tensor_sub(fp_[:], hp1[:], e1[:])
                        if UV_BCAST:
                            uv2 = fe2[:, 0 : 2 * BL].rearrange(
                                "p (a o) -> p a o", a=2
                            )
                            ub = bass.broadcast_tensor_aps(
                                uu.rearrange("p (a o) -> p a o", a=1), uv2
                            )[0]
                            vb = bass.broadcast_tensor_aps(
                                v[:].rearrange("p (a o) -> p a o", a=1), uv2
                            )[0]
                            nc.vector.tensor_mul(uv2, ub, vb)
                            nc.vector.scalar_tensor_tensor(
                                hp1[:], uv, 2.0, fp_[:],
                                op0=OPM, op1=mybir.AluOpType.add,
                            )
                        else:
                            nc.vector.scalar_tensor_tensor(
                                uv, uu, 2.0, v[:], op0=OPM, op1=OPM
                            )
                            nc.vector.tensor_add(hp1[:], fp_[:], uv)

                ph0 = preload(0)
                steps(ph0, 0, min(8, sph))
                ph1 = preload(1)
                steps(ph0, 8, max(8, sph))
                steps(ph1, 0, sph)
                # timing-only long builds overrun hall: write a fixed slice
                hsl = bass.ts(cv, BL) if ncols_eff <= C else slice(0, BL)
                nc.vector.tensor_scalar_add(hall[:, hsl], hp1[:], -1.0)

            with (
                tc.tile_pool(name="col", bufs=2) as colp,
                tc.tile_pool(name="ps", bufs=2, space="PSUM") as psp,
                tc.For_i(
                    0, ncols_eff, 1,
                    hint_engines=(FP.PE, FP.Activation, FP.DVE),
                ) as cv,
            ):
                column_body(colp, psp, cv)

            # output head: bias/2 + partial logits -> allreduce(fwd,bwd)
            # -> softmax(relu(.))
            with (
                tc.tile_pool(name="fc", bufs=1) as fcp,
                tc.tile_pool(name="psfc", bufs=1, space="PSUM") as psfc,
                tc.tile_pool(name="dramp", bufs=1, space="DRAM") as dp,
            ):
                lps = psfc.tile([128, 8 * O], f32)
                nc.tensor.matmul(lps[:], ones1[:], biasr[:], start=True, stop=False)
                for k in range(8):
                    nc.tensor.matmul(
                        lps[:, k * O : (k + 1) * O],
                        hall[:, k * 128 : (k + 1) * 128],
                        wfcT[:],
                        start=False,
                        stop=(k == 7),
                        skip_group_check=True,
                    )
                lsb = fcp.tile([128, 8 * O], f32)
                nc.scalar.copy(lsb[:], lps[:])
                lloc = dp.tile([C * BL, O], f32)
                lred = dp.tile([C * BL, O], f32)
                nc.sync.dma_start(
                    lloc.rearrange("(k p) o -> p k o", p=128),
                    lsb[:].rearrange("p (k o) -> p k o", k=8),
                )
                if skip_collective:
                    nc.sync.dma_start(lred[:], lloc[:])
                else:
                    nc.gpsimd.collective_compute(
                        "AllReduce",
                        mybir.AluOpType.add,
                        replica_groups=[[0, 4], [1, 5], [2, 6], [3, 7]],
                        ins=[lloc.opt()],
                        outs=[lred.opt()],
                    )
                lsum = fcp.tile([128, 8 * O], f32)
                nc.sync.dma_start(
                    lsum[:].rearrange("p (k o) -> p k o", k=8),
                    lred.rearrange("(k p) o -> p k o", p=128),
                )
                ex = fcp.tile([128, 8 * O], f32)
                nc.scalar.activation(ex[:], lsum[:], AF.Exp)
                # exp(relu(x)) == max(1, exp(x))
                nc.vector.tensor_scalar_max(ex[:], ex[:], 1.0)
                sums = fcp.tile([128, 8], f32)
                nc.vector.tensor_reduce(
                    sums[:],
                    ex[:].rearrange("p (k o) -> p k o", k=8),
                    axis=mybir.AxisListType.X,
                    op=mybir.AluOpType.add,
                )
                rs = fcp.tile([128, 8], f32)
                nc.vector.reciprocal(rs[:], sums[:])
                osb = fcp.tile([128, 8 * O], f32)
                for k in range(8):
                    nc.vector.tensor_scalar_mul(
                        osb[:, k * O : (k + 1) * O],
                        ex[:, k * O : (k + 1) * O],
                        rs[:, k : k + 1],
                    )
                nc.sync.dma_start(
                    out_d.rearrange("(k p) o -> p k o", p=128),
                    osb[:].rearrange("p (k o) -> p k o", k=8),
                )


_CACHE = {}


def _build(loop_cols=None, n_cols=C):
    key = ("nc", loop_cols, n_cols)
    if key not in _CACHE:
        nc = bacc.Bacc("TRN2", target_bir_lowering=False, debug=False,
                       num_devices=NCORES)
        _emit(nc, n_cols=n_cols, loop_cols=loop_cols)
        nc.compile()
        _CACHE[key] = nc
    return _CACHE[key]


def _runner(nc, cache_key):
    """Jitted shard_map callable, built once per nc."""
    key = ("run", cache_key)
    if key in _CACHE:
        return _CACHE[key]
    install_neuronx_cc_hook()
    partition_name = nc.partition_id_tensor.name if nc.partition_id_tensor else None
    in_names, out_names, out_avals, zero_outs = [], [], [], []
    for alloc in nc.m.functions[0].allocations:
        if not isinstance(alloc, mybir.MemoryLocationSet):
            continue
        name = alloc.memorylocations[0].name
        if alloc.kind == "ExternalInput":
            if name != partition_name:
                in_names.append(name)
        elif alloc.kind == "ExternalOutput":
            shape = tuple(alloc.tensor_shape)
            dtype = mybir.dt.np(alloc.dtype)
            out_names.append(name)
            out_avals.append(jax.core.ShapedArray(shape, dtype))
            zero_outs.append(np.zeros(shape, dtype))
    n_params = len(in_names)
    n_outs = len(out_avals)
    all_in_names = list(in_names) + list(out_names)
    if partition_name is not None:
        all_in_names.append(partition_name)
    donate = tuple(range(n_params, n_params + n_outs))

    def _body(*args):
        operands = list(args)
        if partition_name is not None:
            operands.append(partition_id_tensor())
        outs = _bass_exec_p.bind(
            *operands,
            out_avals=tuple(out_avals),
            in_names=tuple(all_in_names),
            out_names=tuple(out_names),
            lowering_input_output_aliases=(),
            sim_require_finite=True,
            sim_require_nnan=True,
            nc=nc,
        )
        return tuple(outs)

    devices = jax.devices()[:NCORES]
    mesh = Mesh(np.asarray(devices), ("core",))
    in_specs = (PartitionSpec("core"),) * (n_params + n_outs)
    out_specs = (PartitionSpec("core"),) * n_outs
    jitted = jax.jit(
        shard_map(_body, mesh=mesh, in_specs=in_specs, out_specs=out_specs,
                  check_rep=False),
        donate_argnums=donate,
        keep_unused=True,
    )
    sharding = jax.sharding.NamedSharding(mesh, PartitionSpec("core"))
    dev_cache = {}

    def run(in_maps, device_resident=False):
        if device_resident:
            if "in" not in dev_cache:
                concat = [
                    np.concatenate([np.asarray(m[name]) for m in in_maps], axis=0)
                    for name in in_names
                ]
                dev_cache["in"] = [jax.device_put(a, sharding) for a in concat]
                for a in dev_cache["in"]:
                    a.block_until_ready()
            concat_in = dev_cache["in"]
        else:
            concat_in = [
                np.concatenate([np.asarray(m[name]) for m in in_maps], axis=0)
                for name in in_names
            ]
        concat_zeros = [
            np.zeros((NCORES * z.shape[0], *z.shape[1:]), z.dtype)
            for z in zero_outs
        ]
        out_arrs = jitted(*concat_in, *concat_zeros)
        return [
            {
                name: np.asarray(out_arrs[i]).reshape(NCORES, *out_avals[i].shape)[c]
                for i, name in enumerate(out_names)
            }
            for c in range(NCORES)
        ]

    _CACHE[key] = run
    return run


def _core_inputs(inputs, d, g):
    """Host-side prep for core (direction d, batch group g)."""
    import ml_dtypes

    bsl = slice(g * BL, (g + 1) * BL)
    x = inputs["x"][bsl]
    if d == 1:
        x = x[:, ::-1, :]
    xT = np.ascontiguousarray(np.transpose(x, (2, 1, 0)))  # (C, S, BL)
    xcols = xT.reshape(C, SB)
    xaug = np.empty((C * 2, SB), np.float32)
    xaug[0::2] = xcols
    xaug[1::2] = 1.0
    sfx = "f" if d == 0 else "b"
    Wih = inputs[f"Wih_{sfx}"][:, 0]
    Whh = inputs[f"Whh_{sfx}"]
    bih = inputs[f"bih_{sfx}"]
    bhh = inputs[f"bhh_{sfx}"]
    Wr, Wz, Wn = Whh[:H], Whh[H : 2 * H], Whh[2 * H :]
    lcat = np.zeros((2, 4 * H), np.float32)
    lcat[0, 0:H] = Wih[:H]
    lcat[1, 0:H] = bih[:H] + bhh[:H] - Wr.sum(1)
    lcat[0, H : 2 * H] = Wih[H : 2 * H]
    lcat[1, H : 2 * H] = bih[H : 2 * H] + bhh[H : 2 * H] - Wz.sum(1)
    lcat[1, 2 * H : 3 * H] = bhh[2 * H :] - Wn.sum(1)
    lcat[0, 3 * H : 4 * H] = Wih[2 * H :]
    lcat[1, 3 * H : 4 * H] = bih[2 * H :]
    wfc_half = inputs["W_fc"][:, :H] if d == 0 else inputs["W_fc"][:, H:]
    bias_half = np.tile(0.5 * inputs["b_fc"], 8)[None, :].astype(np.float32)
    return {
        "xaug": xaug.astype(ml_dtypes.bfloat16),
        "hp10": np.ascontiguousarray((inputs["h_prev"][d, bsl] + 1.0).T).astype(
            np.float32
        ),
        "whhrT": np.ascontiguousarray(Wr.T).astype(ml_dtypes.bfloat16),
        "whhzT": np.ascontiguousarray(Wz.T).astype(ml_dtypes.bfloat16),
        "whhnT": np.ascontiguousarray(Wn.T).astype(ml_dtypes.bfloat16),
        "lcat": lcat.astype(ml_dtypes.bfloat16),
        "wfcT": np.ascontiguousarray(wfc_half.T).astype(np.float32),
        "bias_half": bias_half,
    }


def kernel(**inputs) -> np.ndarray:
    inputs = {k: np.asarray(v, dtype=np.float32) for k, v in inputs.items()}
    nc = _build()
    run = _runner(nc, None)
    in_maps = []
    for core in range(NCORES):
        d, g = (0, core) if core < 4 else (1, core - 4)
        in_maps.append(_core_inputs(inputs, d, g))
    res = run(in_maps)
    out = np.empty((B, C, O), np.float32)
    for g in range(4):
        o = res[g]["out"].reshape(C, BL, O)
        out[g * BL : (g + 1) * BL] = np.transpose(o, (1, 0, 2))
    return out


# revision 4
# speedup vs baseline: 2.0422x; 2.0422x over previous
"""Bidirectional column-chained GRU (vertical BiGRU over image columns) on 8 Trainium2 cores.

Topology: cores 0-3 run the forward GRU chain (batch quarters), cores 4-7 the
backward chain (rows pre-reversed on host). Each core runs the full C*S=16384
sequential GRU steps for its 8 batch rows in feature-major layout (128
partitions = hidden dim, free dim = batch).

Math restructuring (validated vs reference):
  state hp1 = h + 1; tanh(x) = 2*sigmoid(2x) - 1 (single ACT table).
  Recurrent weights, input columns and the state pair are bf16 (PSUM
  accumulation stays fp32); the serialized fp32 LDWEIGHTS cost dominated the
  per-step critical cycle, and bf16 enables fast weight load.
  Per column c, each gate's rank-1 input contribution is preloaded into PSUM
  with K=2 matmuls; the recurrent matmul accumulates per step into PSUM
  slice t. Per step:
    r  = sigmoid(ps_r[t])                ACT (PSUM src)
    u  = sigmoid(-ps_z[t])  (= 1-z)      ACT
    q  = r * ps_n[t]                     DVE
    w  = q + a_n[t]                      DVE
    v  = sigmoid(2w)                     ACT
    e1 = u * hp1; f = hp1 - e1           DVE (slack path)
    u2 = 2*u                             DVE (slack path)
    e2 = u2*v                            DVE (plain tensor_tensor; a cycle
                                         stt measured ~190ns/step slower)
    hp1' = f + e2                        DVE (slack path)
  The state pair is stored as [f | e2]; the next step's three matmuls consume
  it directly (PSUM accumulation computes W @ (f + e2) = W @ hp1'), so the
  matmuls never wait for the hp1 update.
  Final per-column features h = hp1 - 1 are collected; the output head
  (fc + relu + softmax) runs on-device with a pairwise AllReduce between the
  fwd/bwd core of each batch group; b_fc/2 is folded into each core's PSUM
  accumulation via a ones-row matmul. exp(relu(x)) == max(1, exp(x)).

Runner: the jitted shard_map callable is built once and cached; repeat
kernel() calls only pay input transfer + execution.
"""

import numpy as np

import jax
from jax.sharding import Mesh, PartitionSpec
from jax.experimental.shard_map import shard_map

import concourse.bass as bass
import concourse.bacc as bacc
import concourse.mybir as mybir
import concourse.tile as tile
from concourse.bass2jax import (
    _bass_exec_p,
    install_neuronx_cc_hook,
    partition_id_tensor,
)

B, S, C, H, O = 32, 128, 128, 128, 64
NCORES = 8
BL = B // 4          # batch rows per core (4 groups x 2 directions)
SB = S * BL          # rhs columns per image column
HS = SB // 2         # half-column psum width (one bank)
NSTEP = S // 2       # steps per half
f32 = mybir.dt.float32
bf16 = mybir.dt.bfloat16
FP = mybir.EngineType

UV_BCAST = False     # measured: [f|e2]+stt beats [uv|uv|f] broadcast-write


def _emit(nc: bacc.Bacc, n_cols: int = C, loop_cols: int | None = None,
          skip_collective: bool = False, sph: int = NSTEP):
    AF = mybir.ActivationFunctionType
    OPM = mybir.AluOpType.mult
    NSL = 3 if UV_BCAST else 2

    xaug_d = nc.dram_tensor("xaug", [n_cols * 2, SB], bf16, kind="ExternalInput").ap()
    hp10_d = nc.dram_tensor("hp10", [H, BL], f32, kind="ExternalInput").ap()
    whhrT_d = nc.dram_tensor("whhrT", [H, H], bf16, kind="ExternalInput").ap()
    whhzT_d = nc.dram_tensor("whhzT", [H, H], bf16, kind="ExternalInput").ap()
    whhnT_d = nc.dram_tensor("whhnT", [H, H], bf16, kind="ExternalInput").ap()
    lcat_d = nc.dram_tensor("lcat", [2, 4 * H], bf16, kind="ExternalInput").ap()
    wfcT_d = nc.dram_tensor("wfcT", [H, O], f32, kind="ExternalInput").ap()
    bias_d = nc.dram_tensor("bias_half", [1, 8 * O], f32, kind="ExternalInput").ap()
    out_d = nc.dram_tensor("out", [C * BL, O], f32, kind="ExternalOutput").ap()

    ncols_eff = n_cols if loop_cols is None else loop_cols

    with tile.TileContext(nc) as tc:
        with tc.tile_pool(name="const", bufs=1) as cp:
            whhrT = cp.tile([H, H], bf16)
            whhzT = cp.tile([H, H], bf16)
            whhnT = cp.tile([H, H], bf16)
            lcat = cp.tile([2, 4 * H], bf16)
            wfcT = cp.tile([H, O], f32)
            biasr = cp.tile([1, 8 * O], f32)
            ones1 = cp.tile([1, H], f32)
            hp1 = cp.tile([H, BL], f32)
            hall = cp.tile([H, C * BL], f32)
            r = cp.tile([H, BL], f32)
            u = cp.tile([H, BL], f32)
            u2t = cp.tile([H, BL], f32)
            q = cp.tile([H, BL], f32)
            w = cp.tile([H, BL], f32)
            v = cp.tile([H, BL], f32)
            e1 = cp.tile([H, BL], f32)
            fe2 = cp.tile([H, NSL * BL], bf16)
            if UV_BCAST:
                uv = fe2[:, 0:BL]
                fp_ = fe2[:, 2 * BL : 3 * BL]
            else:
                fp_ = fe2[:, 0:BL]
                uv = fe2[:, BL : 2 * BL]

            nc.sync.dma_start(whhrT[:], whhrT_d)
            nc.sync.dma_start(whhzT[:], whhzT_d)
            nc.sync.dma_start(whhnT[:], whhnT_d)
            nc.sync.dma_start(lcat[:], lcat_d)
            nc.sync.dma_start(wfcT[:], wfcT_d)
            nc.sync.dma_start(biasr[:], bias_d)
            nc.sync.dma_start(hp1[:], hp10_d)
            nc.vector.memset(ones1[:], 1.0)
            nc.vector.tensor_scalar_add(fp_[:], hp1[:], 0.0)  # f32->bf16
            if UV_BCAST:
                nc.vector.memzero(fe2[:, 0 : 2 * BL])
            else:
                nc.vector.memzero(uv)

            def column_body(colp, psp, cv):
                xa = colp.tile([2, SB], bf16, tag="xa")
                nc.sync.dma_start(xa[:], xaug_d[bass.ds(cv * 2, 2), :])

                def preload(half):
                    ps_r = psp.tile([H, HS], f32, tag="ps_r", name=f"ps_r{half}")[:]
                    ps_z = psp.tile([H, HS], f32, tag="ps_z", name=f"ps_z{half}")[:]
                    ps_n = psp.tile([H, HS], f32, tag="ps_n", name=f"ps_n{half}")
                    ps_t = psp.tile([H, HS], f32, tag="ps_t", name=f"ps_t{half}")
                    a_n = colp.tile([H, HS], f32, tag="a_n", name=f"a_n{half}")
                    xh = xa[:, half * HS : (half + 1) * HS]
                    lcv = lcat[:]
                    nc.tensor.matmul(ps_r, lcv[:, 0:H], xh, start=True, stop=True)
                    nc.tensor.matmul(ps_z, lcv[:, H : 2 * H], xh, start=True, stop=True)
                    nc.tensor.matmul(ps_n[:], lcv[:, 2 * H : 3 * H], xh, start=True, stop=True)
                    nc.tensor.matmul(ps_t[:], lcv[:, 3 * H : 4 * H], xh, start=True, stop=True)
                    nc.scalar.copy(a_n[:], ps_t[:])
                    return ps_r, ps_z, ps_n, a_n

                def steps(ph, lo, hi):
                    ps_r, ps_z, ps_n, a_n = ph
                    for t in range(lo, hi):
                        sl = slice(t * BL, (t + 1) * BL)
                        hp1v = fe2[:].rearrange("p (a o) -> p a o", a=NSL)
                        mm_order = [(ps_r, whhrT), (ps_n[:], whhnT), (ps_z, whhzT)]
                        outs = [
                            bass.broadcast_tensor_aps(
                                ps[:, sl].rearrange("p (a o) -> p a o", a=1),
                                hp1v,
                            )[0]
                            for ps, _ in mm_order
                        ]
                        for o_, (_, w_) in zip(outs, mm_order):
                            nc.tensor.matmul(
                                o_, w_[:], hp1v, start=False, stop=True,
                                skip_group_check=True,
                            )
                        nc.scalar.activation(r[:], ps_r[:, sl], AF.Sigmoid)
                        nc.scalar.activation(u[:], ps_z[:, sl], AF.Sigmoid, scale=-1.0)
                        rr = r[:]
                        uu = u[:]
                        nc.vector.tensor_scalar_mul(u2t[:], uu, 2.0)
                        nc.vector.tensor_mul(q[:], rr, ps_n[:, sl])
                        nc.vector.tensor_add(w[:], q[:], a_n[:, sl])
                        nc.scalar.activation(v[:], w[:], AF.Sigmoid, scale=2.0)
                        nc.vector.tensor_mul(e1[:], uu, hp1[:])
                        nc.vector.tensor_sub(fp_[:], hp1[:], e1[:])
                        if UV_BCAST:
                            uv2 = fe2[:, 0 : 2 * BL].rearrange(
                                "p (a o) -> p a o", a=2
                            )
                            ub = bass.broadcast_tensor_aps(
                                uu.rearrange("p (a o) -> p a o", a=1), uv2
                            )[0]
                            vb = bass.broadcast_tensor_aps(
                                v[:].rearrange("p (a o) -> p a o", a=1), uv2
                            )[0]
                            nc.vector.tensor_mul(uv2, ub, vb)
                            nc.vector.scalar_tensor_tensor(
                                hp1[:], uv, 2.0, fp_[:],
                                op0=OPM, op1=mybir.AluOpType.add,
                            )
                        else:
                            nc.vector.tensor_mul(uv, u2t[:], v[:])
                            nc.vector.tensor_add(hp1[:], fp_[:], uv)

                ph0 = preload(0)
                steps(ph0, 0, min(8, sph))
                ph1 = preload(1)
                steps(ph0, 8, max(8, sph))
                steps(ph1, 0, sph)
                # timing-only long builds overrun hall: write a fixed slice
                hsl = bass.ts(cv, BL) if ncols_eff <= C else slice(0, BL)
                nc.vector.tensor_scalar_add(hall[:, hsl], hp1[:], -1.0)

            with (
                tc.tile_pool(name="col", bufs=2) as colp,
                tc.tile_pool(name="ps", bufs=2, space="PSUM") as psp,
                tc.For_i(
                    0, ncols_eff, 1,
                    hint_engines=(FP.PE, FP.Activation, FP.DVE),
                ) as cv,
            ):
                column_body(colp, psp, cv)

            # output head: bias/2 + partial logits -> allreduce(fwd,bwd)
            # -> softmax(relu(.))
            with (
                tc.tile_pool(name="fc", bufs=1) as fcp,
                tc.tile_pool(name="psfc", bufs=1, space="PSUM") as psfc,
                tc.tile_pool(name="dramp", bufs=1, space="DRAM") as dp,
            ):
                lps = psfc.tile([128, 8 * O], f32)
                nc.tensor.matmul(lps[:], ones1[:], biasr[:], start=True, stop=False)
                for k in range(8):
                    nc.tensor.matmul(
                        lps[:, k * O : (k + 1) * O],
                        hall[:, k * 128 : (k + 1) * 128],
                        wfcT[:],
                        start=False,
                        stop=(k == 7),
                        skip_group_check=True,
                    )
                lsb = fcp.tile([128, 8 * O], f32)
                nc.scalar.copy(lsb[:], lps[:])
                lloc = dp.tile([C * BL, O], f32)
                lred = dp.tile([C * BL, O], f32)
                nc.sync.dma_start(
                    lloc.rearrange("(k p) o -> p k o", p=128),
                    lsb[:].rearrange("p (k o) -> p k o", k=8),
                )
                if skip_collective:
                    nc.sync.dma_start(lred[:], lloc[:])
                else:
                    nc.gpsimd.collective_compute(
                        "AllReduce",
                        mybir.AluOpType.add,
                        replica_groups=[[0, 4], [1, 5], [2, 6], [3, 7]],
                        ins=[lloc.opt()],
                        outs=[lred.opt()],
                    )
                lsum = fcp.tile([128, 8 * O], f32)
                nc.sync.dma_start(
                    lsum[:].rearrange("p (k o) -> p k o", k=8),
                    lred.rearrange("(k p) o -> p k o", p=128),
                )
                ex = fcp.tile([128, 8 * O], f32)
                nc.scalar.activation(ex[:], lsum[:], AF.Exp)
                # exp(relu(x)) == max(1, exp(x))
                nc.vector.tensor_scalar_max(ex[:], ex[:], 1.0)
                sums = fcp.tile([128, 8], f32)
                nc.vector.tensor_reduce(
                    sums[:],
                    ex[:].rearrange("p (k o) -> p k o", k=8),
                    axis=mybir.AxisListType.X,
                    op=mybir.AluOpType.add,
                )
                rs = fcp.tile([128, 8], f32)
                nc.vector.reciprocal(rs[:], sums[:])
                osb = fcp.tile([128, 8 * O], f32)
                for k in range(8):
                    nc.vector.tensor_scalar_mul(
                        osb[:, k * O : (k + 1) * O],
                        ex[:, k * O : (k + 1) * O],
                        rs[:, k : k + 1],
                    )
                nc.sync.dma_start(
                    out_d.rearrange("(k p) o -> p k o", p=128),
                    osb[:].rearrange("p (k o) -> p k o", k=8),
                )


_CACHE = {}


def _build(loop_cols=None, n_cols=C):
    key = ("nc", loop_cols, n_cols)
    if key not in _CACHE:
        nc = bacc.Bacc("TRN2", target_bir_lowering=False, debug=False,
                       num_devices=NCORES)
        _emit(nc, n_cols=n_cols, loop_cols=loop_cols)
        nc.compile()
        _CACHE[key] = nc
    return _CACHE[key]


def _runner(nc, cache_key):
    """Jitted shard_map callable, built once per nc."""
    key = ("run", cache_key)
    if key in _CACHE:
        return _CACHE[key]
    install_neuronx_cc_hook()
    partition_name = nc.partition_id_tensor.name if nc.partition_id_tensor else None
    in_names, out_names, out_avals, zero_outs = [], [], [], []
    for alloc in nc.m.functions[0].allocations:
        if not isinstance(alloc, mybir.MemoryLocationSet):
            continue
        name = alloc.memorylocations[0].name
        if alloc.kind == "ExternalInput":
            if name != partition_name:
                in_names.append(name)
        elif alloc.kind == "ExternalOutput":
            shape = tuple(alloc.tensor_shape)
            dtype = mybir.dt.np(alloc.dtype)
            out_names.append(name)
            out_avals.append(jax.core.ShapedArray(shape, dtype))
            zero_outs.append(np.zeros(shape, dtype))
    n_params = len(in_names)
    n_outs = len(out_avals)
    all_in_names = list(in_names) + list(out_names)
    if partition_name is not None:
        all_in_names.append(partition_name)
    donate = tuple(range(n_params, n_params + n_outs))

    def _body(*args):
        operands = list(args)
        if partition_name is not None:
            operands.append(partition_id_tensor())
        outs = _bass_exec_p.bind(
            *operands,
            out_avals=tuple(out_avals),
            in_names=tuple(all_in_names),
            out_names=tuple(out_names),
            lowering_input_output_aliases=(),
            sim_require_finite=True,
            sim_require_nnan=True,
            nc=nc,
        )
        return tuple(outs)

    devices = jax.devices()[:NCORES]
    mesh = Mesh(np.asarray(devices), ("core",))
    in_specs = (PartitionSpec("core"),) * (n_params + n_outs)
    out_specs = (PartitionSpec("core"),) * n_outs
    jitted = jax.jit(
        shard_map(_body, mesh=mesh, in_specs=in_specs, out_specs=out_specs,
                  check_rep=False),
        donate_argnums=donate,
        keep_unused=True,
    )
    sharding = jax.sharding.NamedSharding(mesh, PartitionSpec("core"))
    dev_cache = {}

    def run(in_maps, device_resident=False):
        if device_resident:
            if "in" not in dev_cache:
                concat = [
                    np.concatenate([np.asarray(m[name]) for m in in_maps], axis=0)
                    for name in in_names
                ]
                dev_cache["in"] = [jax.device_put(a, sharding) for a in concat]
                for a in dev_cache["in"]:
                    a.block_until_ready()
            concat_in = dev_cache["in"]
        else:
            concat_in = [
                np.concatenate([np.asarray(m[name]) for m in in_maps], axis=0)
                for name in in_names
            ]
        concat_zeros = [
            np.zeros((NCORES * z.shape[0], *z.shape[1:]), z.dtype)
            for z in zero_outs
        ]
        out_arrs = jitted(*concat_in, *concat_zeros)
        return [
            {
                name: np.asarray(out_arrs[i]).reshape(NCORES, *out_avals[i].shape)[c]
                for i, name in enumerate(out_names)
            }
            for c in range(NCORES)
        ]

    _CACHE[key] = run
    return run


def _core_inputs(inputs, d, g):
    """Host-side prep for core (direction d, batch group g)."""
    import ml_dtypes

    bsl = slice(g * BL, (g + 1) * BL)
    x = inputs["x"][bsl]
    if d == 1:
        x = x[:, ::-1, :]
    xT = np.ascontiguousarray(np.transpose(x, (2, 1, 0)))  # (C, S, BL)
    xcols = xT.reshape(C, SB)
    xaug = np.empty((C * 2, SB), np.float32)
    xaug[0::2] = xcols
    xaug[1::2] = 1.0
    sfx = "f" if d == 0 else "b"
    Wih = inputs[f"Wih_{sfx}"][:, 0]
    Whh = inputs[f"Whh_{sfx}"]
    bih = inputs[f"bih_{sfx}"]
    bhh = inputs[f"bhh_{sfx}"]
    Wr, Wz, Wn = Whh[:H], Whh[H : 2 * H], Whh[2 * H :]
    lcat = np.zeros((2, 4 * H), np.float32)
    lcat[0, 0:H] = Wih[:H]
    lcat[1, 0:H] = bih[:H] + bhh[:H] - Wr.sum(1)
    lcat[0, H : 2 * H] = Wih[H : 2 * H]
    lcat[1, H : 2 * H] = bih[H : 2 * H] + bhh[H : 2 * H] - Wz.sum(1)
    lcat[1, 2 * H : 3 * H] = bhh[2 * H :] - Wn.sum(1)
    lcat[0, 3 * H : 4 * H] = Wih[2 * H :]
    lcat[1, 3 * H : 4 * H] = bih[2 * H :]
    wfc_half = inputs["W_fc"][:, :H] if d == 0 else inputs["W_fc"][:, H:]
    bias_half = np.tile(0.5 * inputs["b_fc"], 8)[None, :].astype(np.float32)
    return {
        "xaug": xaug.astype(ml_dtypes.bfloat16),
        "hp10": np.ascontiguousarray((inputs["h_prev"][d, bsl] + 1.0).T).astype(
            np.float32
        ),
        "whhrT": np.ascontiguousarray(Wr.T).astype(ml_dtypes.bfloat16),
        "whhzT": np.ascontiguousarray(Wz.T).astype(ml_dtypes.bfloat16),
        "whhnT": np.ascontiguousarray(Wn.T).astype(ml_dtypes.bfloat16),
        "lcat": lcat.astype(ml_dtypes.bfloat16),
        "wfcT": np.ascontiguousarray(wfc_half.T).astype(np.float32),
        "bias_half": bias_half,
    }


def kernel(**inputs) -> np.ndarray:
    inputs = {k: np.asarray(v, dtype=np.float32) for k, v in inputs.items()}
    nc = _build()
    run = _runner(nc, None)
    in_maps = []
    for core in range(NCORES):
        d, g = (0, core) if core < 4 else (1, core - 4)
        in_maps.append(_core_inputs(inputs, d, g))
    res = run(in_maps)
    out = np.empty((B, C, O), np.float32)
    for g in range(4):
        o = res[g]["out"].reshape(C, BL, O)
        out[g * BL : (g + 1) * BL] = np.transpose(o, (1, 0, 2))
    return out
